# revision 4
# baseline (speedup 1.0000x reference)
"""ConvCaps EM-routing kernel for Trainium2 (8 NeuronCores, data-parallel over N).

Strategy:
  - N = B*Ho*Wo = 4096 positions; shard 512 per core (2 batch images), processed
    in 4 chunks of 128 (chunk = one partition-dim tile, n on partitions).
  - vote[n, c, p, m] (m = 4x4 pose components) built on the TensorEngine via a
    block-diagonal weight matrix: per pose-column k, out[n, (c,p,i)] =
    poseT_k[(p,j), n].T @ A[(p,j), (c,p,i)], A = W * delta(p) / 9 (pool scale
    folded in). pose pooling = 9 shifted DMA loads + adds.
  - 3 EM iterations fully unrolled on-chip: weighted stats (S1, S2) via DVE
    multiply + strided-axis reduces; transcendentals on ScalarE using only the
    ln/exp table set (sqrt = exp(0.5 ln), rsqrt = exp(-0.5 ln),
    sigmoid = 1/(1+exp(-x))); E-step skipped on the last iteration.
"""

import os
import sys
import numpy as np

sys.path.insert(0, "/opt/trn_rl_repo")

import concourse.bass as bass
import concourse.bacc as bacc
import concourse.mybir as mybir
from concourse.tile import TileContext
from concourse.bass_utils import run_bass_kernel_spmd

F32 = mybir.dt.float32
Alu = mybir.AluOpType
Act = mybir.ActivationFunctionType
AxX = mybir.AxisListType.X

B, H, W, CP, CN = 16, 18, 18, 32, 32
KH, KW = 3, 3
Ho, Wo = H - KH + 1, W - KW + 1  # 16, 16
Ntot = B * Ho * Wo               # 4096
NCORES = 8
NLOC = Ntot // NCORES            # 512
IMGS = B // NCORES               # 2 images per core
CHUNKS = NLOC // 128             # 4
M = 16                           # pose components per capsule
CPM = CP * M                     # 512
CNM = CN * M                     # 512
VSZ = CN * CP * M                # 16384


def _build(inv_temp: float, pi: float):
    nc = bacc.Bacc(None, target_bir_lowering=False, debug=False, num_devices=NCORES)

    poseR = nc.dram_tensor("poseR", [IMGS, H, W, CPM], F32, kind="ExternalInput").ap()
    actR = nc.dram_tensor("actR", [IMGS, H, W, CP], F32, kind="ExternalInput").ap()
    Abd = nc.dram_tensor("Abd", [128, CN * CP * 4], F32, kind="ExternalInput").ap()
    bvD = nc.dram_tensor("bvD", [128, CNM], F32, kind="ExternalInput").ap()
    baD = nc.dram_tensor("baD", [128, CN], F32, kind="ExternalInput").ap()
    idD = nc.dram_tensor("idD", [128, 128], F32, kind="ExternalInput").ap()
    meanO = nc.dram_tensor("meanO", [NLOC, CNM], F32, kind="ExternalOutput").ap()
    actO = nc.dram_tensor("actO", [NLOC, CN], F32, kind="ExternalOutput").ap()

    with TileContext(nc) as tc:
        with tc.tile_pool(name="const", bufs=1) as cpool, \
             tc.tile_pool(name="big", bufs=1) as bigp, \
             tc.tile_pool(name="pose", bufs=2) as posep, \
             tc.tile_pool(name="sm", bufs=1) as smp, \
             tc.tile_pool(name="psT", bufs=2, space="PSUM") as psT, \
             tc.tile_pool(name="psMM", bufs=3, space="PSUM") as psMM:

            Abd_t = cpool.tile([128, CN * CP * 4], F32)
            nc.sync.dma_start(Abd_t[:], Abd[:])
            bv_t = cpool.tile([128, CNM], F32)
            nc.sync.dma_start(bv_t[:], bvD[:])
            ba_t = cpool.tile([128, CN], F32)
            nc.sync.dma_start(ba_t[:], baD[:])
            ident = cpool.tile([128, 128], F32)
            nc.sync.dma_start(ident[:], idD[:])
            bias5 = cpool.tile([128, 1], F32)
            nc.vector.memset(bias5[:], 1e-5)
            bias6 = cpool.tile([128, 1], F32)
            nc.vector.memset(bias6[:], 1e-6)

            for chunk in range(CHUNKS):
                ib, ih = chunk // 2, chunk % 2
                h0 = 8 * ih

                # ---- pose pooling: acc[n, (p,j,k)] = sum of 9 shifted windows
                acc = posep.tile([128, CPM], F32, tag="acc")
                for t, (dh, dw) in enumerate((dh, dw) for dh in range(3) for dw in range(3)):
                    src = poseR[ib, h0 + dh:h0 + dh + 8, dw:dw + 16, :]
                    if t == 0:
                        nc.sync.dma_start(acc[:], src)
                    else:
                        pl = posep.tile([128, CPM], F32, tag="pl")
                        nc.sync.dma_start(pl[:], src)
                        nc.vector.tensor_tensor(acc[:], acc[:], pl[:], Alu.add)

                # ---- act pooling (scale 1/9 applied -> a_in; R0 = a_in/CN)
                acca = posep.tile([128, CP], F32, tag="acca")
                for t, (dh, dw) in enumerate((dh, dw) for dh in range(3) for dw in range(3)):
                    src = actR[ib, h0 + dh:h0 + dh + 8, dw:dw + 16, :]
                    if t == 0:
                        nc.sync.dma_start(acca[:], src)
                    else:
                        al = posep.tile([128, CP], F32, tag="al")
                        nc.sync.dma_start(al[:], src)
                        nc.vector.tensor_tensor(acca[:], acca[:], al[:], Alu.add)
                a_in = smp.tile([128, CP], F32, tag="a_in")
                nc.vector.tensor_scalar_mul(a_in[:], acca[:], 1.0 / 9.0)
                R0 = smp.tile([128, CP], F32, tag="R0")
                nc.vector.tensor_scalar_mul(R0[:], acca[:], 1.0 / (9.0 * CN))

                # ---- poseT[(p,j), n] per k  (gather k-slice, PE transpose, evict)
                poseT = posep.tile([128, 512], F32, tag="poseT")
                acc_v = acc[:].rearrange("n (pj k) -> n k pj", k=4)
                for k in range(4):
                    ks = posep.tile([128, 128], F32, tag="ks")
                    nc.vector.tensor_copy(ks[:], acc_v[:, k, :])
                    pst = psT.tile([128, 128], F32)
                    nc.tensor.transpose(pst[:], ks[:], ident[:])
                    nc.scalar.copy(poseT[:, 128 * k:128 * (k + 1)], pst[:])

                # ---- vote via block-diagonal matmuls
                vote = bigp.tile([128, VSZ], F32, tag="vote")
                vote_r = vote[:].rearrange("n (c p i k) -> n c p i k", c=CN, p=CP, i=4)
                for k in range(4):
                    for half in range(4):  # 2 rhs chunks of 512 per psum tile
                        pm = psMM.tile([128, 1024], F32)
                        for q in range(2):
                            cg = half * 2 + q
                            nc.tensor.matmul(
                                pm[:, 512 * q:512 * (q + 1)],
                                poseT[:, 128 * k:128 * (k + 1)],
                                Abd_t[:, 512 * cg:512 * (cg + 1)],
                                start=True, stop=True)
                        c0 = half * 8
                        nc.scalar.copy(
                            vote_r[:, c0:c0 + 8, :, :, k],
                            pm[:].rearrange("n (c p i) -> n c p i", c=8, p=CP))

                vote_v = vote[:].rearrange("n (c p m) -> n c p m", c=CN, p=CP)
                scr = bigp.tile([128, VSZ], F32, tag="scr")
                scr_v = scr[:].rearrange("n (c p m) -> n c p m", c=CN, p=CP)

                R = smp.tile([128, CN * CP], F32, tag="R")
                S1 = smp.tile([128, CNM], F32, tag="S1")
                S2 = smp.tile([128, CNM], F32, tag="S2")
                mean = smp.tile([128, CNM], F32, tag="mean")
                var = smp.tile([128, CNM], F32, tag="var")
                t1 = smp.tile([128, CNM], F32, tag="t1")
                t2 = smp.tile([128, CNM], F32, tag="t2")
                b1 = smp.tile([128, CNM], F32, tag="b1")
                sE = smp.tile([128, CN * CP], F32, tag="sE")
                pu = smp.tile([128, CN * CP], F32, tag="pu")
                cost = smp.tile([128, CNM], F32, tag="cost")
                sum_cost = smp.tile([128, CN], F32, tag="sum_cost")
                sd = smp.tile([128, CN], F32, tag="sd")
                sq = smp.tile([128, CN], F32, tag="sq")
                u1 = smp.tile([128, CN], F32, tag="u1")
                act = smp.tile([128, CN], F32, tag="act")
                rsum = smp.tile([128, CN], F32, tag="rsum")
                i5 = smp.tile([128, CN], F32, tag="i5")
                i3 = smp.tile([128, CN], F32, tag="i3")
                varsum = smp.tile([128, CN], F32, tag="varsum")
                denom = smp.tile([128, CN], F32, tag="denom")
                adr = smp.tile([128, CN], F32, tag="adr")
                apsum = smp.tile([128, CP], F32, tag="apsum")
                rra = smp.tile([128, CP], F32, tag="rra")
                sc1 = smp.tile([128, 1], F32, tag="sc1")
                sc2 = smp.tile([128, 1], F32, tag="sc2")
                sc3 = smp.tile([128, 1], F32, tag="sc3")

                for t in range(3):
                    it_val = inv_temp + t
                    # ================= M-step =================
                    if t == 0:
                        Rb = R0[:].unsqueeze(1).unsqueeze(3).broadcast_to([128, CN, CP, M])
                    else:
                        Rb = R[:].rearrange("n (c p) -> n c p", c=CN).unsqueeze(3) \
                                 .broadcast_to([128, CN, CP, M])
                    nc.vector.tensor_tensor(scr_v, vote_v, Rb, Alu.mult)
                    red_v = scr_v.transpose([0, 1, 3, 2])  # [n, c, m, p]
                    nc.vector.tensor_reduce(
                        S1[:].rearrange("n (c m) -> n c m", c=CN), red_v, axis=AxX, op=Alu.add)
                    nc.vector.tensor_tensor(scr[:], scr[:], vote[:], Alu.mult)
                    nc.vector.tensor_reduce(
                        S2[:].rearrange("n (c m) -> n c m", c=CN), red_v, axis=AxX, op=Alu.add)

                    if t == 0:
                        # r_sum identical for all c: per-partition scalar
                        nc.vector.tensor_reduce(sc1[:], R0[:], axis=AxX, op=Alu.add)
                        nc.vector.tensor_scalar_add(sc2[:], sc1[:], 1e-5)
                        nc.vector.reciprocal(sc2[:], sc2[:])  # 1/(rs+1e-5)
                        nc.vector.tensor_scalar_mul(mean[:], S1[:], sc2[:, 0:1])
                        nc.vector.tensor_scalar_add(sc3[:], sc1[:], 1e-3)
                        nc.vector.reciprocal(sc3[:], sc3[:])  # 1/(rs+1e-3)
                        nc.vector.tensor_tensor(t1[:], mean[:], S1[:], Alu.mult)
                        # t2 = (mean*rs)*mean
                        nc.vector.scalar_tensor_tensor(
                            t2[:], mean[:], sc1[:, 0:1], mean[:], Alu.mult, Alu.mult)
                        # var = (S2 - 2*t1 + t2) / (rs+1e-3)
                        nc.vector.scalar_tensor_tensor(
                            t1[:], t1[:], -2.0, S2[:], Alu.mult, Alu.add)
                        nc.vector.tensor_tensor(t1[:], t1[:], t2[:], Alu.add)
                        nc.vector.tensor_scalar_mul(var[:], t1[:], sc3[:, 0:1])
                        rsum_b = None
                    else:
                        nc.vector.tensor_reduce(
                            rsum[:], R[:].rearrange("n (c p) -> n c p", c=CN), axis=AxX, op=Alu.add)
                        nc.vector.tensor_scalar_add(i5[:], rsum[:], 1e-5)
                        nc.vector.reciprocal(i5[:], i5[:])
                        nc.vector.tensor_scalar_add(i3[:], rsum[:], 1e-3)
                        nc.vector.reciprocal(i3[:], i3[:])
                        i5_b = i5[:].unsqueeze(2).broadcast_to([128, CN, M])
                        mean_cm = mean[:].rearrange("n (c m) -> n c m", c=CN)
                        S1_cm = S1[:].rearrange("n (c m) -> n c m", c=CN)
                        S2_cm = S2[:].rearrange("n (c m) -> n c m", c=CN)
                        t1_cm = t1[:].rearrange("n (c m) -> n c m", c=CN)
                        t2_cm = t2[:].rearrange("n (c m) -> n c m", c=CN)
                        var_cm = var[:].rearrange("n (c m) -> n c m", c=CN)
                        rsum_b = rsum[:].unsqueeze(2).broadcast_to([128, CN, M])
                        i3_b = i3[:].unsqueeze(2).broadcast_to([128, CN, M])
                        nc.vector.tensor_tensor(mean_cm, S1_cm, i5_b, Alu.mult)
                        nc.vector.tensor_tensor(t1[:], mean[:], S1[:], Alu.mult)
                        nc.vector.tensor_tensor(t2[:], mean[:], mean[:], Alu.mult)
                        nc.vector.tensor_tensor(t2_cm, t2_cm, rsum_b, Alu.mult)
                        nc.vector.scalar_tensor_tensor(
                            t1[:], t1[:], -2.0, S2[:], Alu.mult, Alu.add)
                        nc.vector.tensor_tensor(t1[:], t1[:], t2[:], Alu.add)
                        nc.vector.tensor_tensor(var_cm, t1_cm, i3_b, Alu.mult)

                    # ---- cost -> act_out
                    nc.scalar.activation(t1[:], var[:], Act.Ln, bias=bias6[:, 0:1])
                    nc.scalar.activation(t2[:], t1[:], Act.Exp, scale=0.5)      # sqrt(var+1e-6)
                    nc.scalar.activation(t1[:], t2[:], Act.Ln, bias=bias5[:, 0:1])      # ln(sqrt+1e-5)
                    nc.vector.tensor_tensor(cost[:], t1[:], bv_t[:], Alu.add)
                    if t == 0:
                        nc.vector.tensor_scalar_mul(cost[:], cost[:], sc1[:, 0:1])
                    else:
                        nc.vector.tensor_tensor(
                            cost[:].rearrange("n (c m) -> n c m", c=CN),
                            cost[:].rearrange("n (c m) -> n c m", c=CN),
                            rsum_b, Alu.mult)
                    nc.vector.tensor_reduce(
                        sum_cost[:], cost[:].rearrange("n (c m) -> n c m", c=CN),
                        axis=AxX, op=Alu.add)
                    nc.vector.tensor_reduce(sc1[:], sum_cost[:], axis=AxX, op=Alu.add)
                    nc.vector.tensor_scalar_mul(sc1[:], sc1[:], 1.0 / CN)       # scm
                    nc.vector.tensor_scalar_sub(sd[:], sum_cost[:], sc1[:, 0:1])
                    nc.vector.tensor_tensor(sq[:], sd[:], sd[:], Alu.mult)
                    nc.vector.tensor_reduce(sc2[:], sq[:], axis=AxX, op=Alu.add)
                    nc.scalar.activation(sc2[:], sc2[:], Act.Ln, scale=1.0 / CN, bias=bias5[:, 0:1])
                    nc.scalar.activation(sc2[:], sc2[:], Act.Exp, scale=0.5)    # std
                    nc.vector.tensor_scalar_add(sc2[:], sc2[:], 1e-5)
                    nc.vector.reciprocal(sc2[:], sc2[:])                        # 1/(std+1e-5)
                    nc.vector.tensor_scalar_mul(u1[:], sd[:], sc2[:, 0:1])
                    nc.vector.tensor_tensor(u1[:], ba_t[:], u1[:], Alu.subtract)
                    nc.scalar.activation(act[:], u1[:], Act.Exp, scale=-it_val)
                    nc.vector.tensor_scalar_add(act[:], act[:], 1.0)
                    nc.vector.reciprocal(act[:], act[:])                        # sigmoid

                    if t == 2:
                        nc.sync.dma_start(meanO[128 * chunk:128 * (chunk + 1), :], mean[:])
                        nc.sync.dma_start(actO[128 * chunk:128 * (chunk + 1), :], act[:])
                        continue

                    # ================= E-step =================
                    nc.scalar.activation(t1[:], var[:], Act.Ln, scale=2.0, bias=bias5[:, 0:1])
                    nc.scalar.activation(b1[:], t1[:], Act.Exp, scale=-0.5)  # rsqrt(2v+1e-5)
                    mean_b = mean[:].rearrange("n (c m) -> n c m", c=CN).unsqueeze(2) \
                                    .broadcast_to([128, CN, CP, M])
                    b1_b = b1[:].rearrange("n (c m) -> n c m", c=CN).unsqueeze(2) \
                                .broadcast_to([128, CN, CP, M])
                    nc.vector.tensor_tensor(scr_v, vote_v, mean_b, Alu.subtract)
                    nc.vector.tensor_tensor(scr_v, scr_v, b1_b, Alu.mult)
                    nc.scalar.activation(scr[:], scr[:], Act.Square)
                    nc.vector.tensor_reduce(
                        sE[:].rearrange("n (c p) -> n c p", c=CN), scr_v, axis=AxX, op=Alu.add)

                    nc.vector.tensor_reduce(
                        varsum[:], var[:].rearrange("n (c m) -> n c m", c=CN),
                        axis=AxX, op=Alu.add)
                    nc.scalar.activation(denom[:], varsum[:], Act.Ln, scale=2.0 * pi)
                    nc.scalar.activation(denom[:], denom[:], Act.Exp, scale=0.5)
                    nc.vector.tensor_scalar_add(denom[:], denom[:], 1e-4)
                    nc.vector.reciprocal(denom[:], denom[:])
                    nc.vector.tensor_tensor(adr[:], act[:], denom[:], Alu.mult)

                    nc.scalar.activation(pu[:], sE[:], Act.Exp, scale=-1.0)
                    adr_b = adr[:].unsqueeze(2).broadcast_to([128, CN, CP])
                    pu_cp = pu[:].rearrange("n (c p) -> n c p", c=CN)
                    nc.vector.tensor_tensor(pu_cp, pu_cp, adr_b, Alu.mult)
                    nc.vector.tensor_reduce(
                        apsum[:], pu_cp.transpose([0, 2, 1]), axis=AxX, op=Alu.add)
                    nc.vector.tensor_scalar_add(apsum[:], apsum[:], 1e-5)
                    nc.vector.reciprocal(apsum[:], apsum[:])
                    nc.vector.tensor_tensor(rra[:], apsum[:], a_in[:], Alu.mult)
                    rra_b = rra[:].unsqueeze(1).broadcast_to([128, CN, CP])
                    nc.vector.tensor_tensor(
                        R[:].rearrange("n (c p) -> n c p", c=CN), pu_cp, rra_b, Alu.mult)

    nc.compile()
    return nc


def kernel(inputPose, inputActivation, weightMatrix, Bv, Ba,
           height, width, epsilon, inverse_temperature, pi):
    inputPose = np.asarray(inputPose, dtype=np.float32)
    inputActivation = np.asarray(inputActivation, dtype=np.float32)
    W0 = np.asarray(weightMatrix, dtype=np.float32)[0]          # [CN, CP, 4, 4]
    bv = np.asarray(Bv, dtype=np.float32)[0, :, 0, :]           # [CN, 16]
    ba = np.asarray(Ba, dtype=np.float32)[0, :, 0]              # [CN]
    inv_temp = float(np.asarray(inverse_temperature))
    pi_f = float(np.asarray(pi))

    # Block-diagonal weights with pooling scale folded in:
    # A[(p,j), (c,p,i)] = W0[c,p,i,j]/9
    A = np.zeros((CP, 4, CN, CP, 4), dtype=np.float32)
    W0t = W0.transpose(1, 3, 0, 2)  # [p, j, c, i]
    idx = np.arange(CP)
    A[idx, :, :, idx, :] = W0t / 9.0
    A = A.reshape(128, CN * CP * 4)

    bv_rep = np.broadcast_to(bv.reshape(1, CNM), (128, CNM)).copy()
    ba_rep = np.broadcast_to(ba.reshape(1, CN), (128, CN)).copy()
    ident = np.eye(128, dtype=np.float32)

    poseF = inputPose.reshape(B, H, W, CPM)
    in_maps = []
    for d in range(NCORES):
        in_maps.append({
            "poseR": poseF[IMGS * d:IMGS * (d + 1)],
            "actR": inputActivation[IMGS * d:IMGS * (d + 1)],
            "Abd": A, "bvD": bv_rep, "baD": ba_rep, "idD": ident,
        })

    nc = _build(inv_temp, pi_f)
    res = run_bass_kernel_spmd(nc, in_maps, list(range(NCORES)))
    mean = np.concatenate([r["meanO"] for r in res.results], axis=0).reshape(Ntot, CN, M)
    actv = np.concatenate([r["actO"] for r in res.results], axis=0)
    return mean, actv


def bench(inputs, reps=10):
    """Build once, run `reps` times through the cached PJRT executable."""
    import time
    inputPose = np.asarray(inputs["inputPose"], dtype=np.float32)
    inputActivation = np.asarray(inputs["inputActivation"], dtype=np.float32)
    W0 = np.asarray(inputs["weightMatrix"], dtype=np.float32)[0]
    bv = np.asarray(inputs["Bv"], dtype=np.float32)[0, :, 0, :]
    ba = np.asarray(inputs["Ba"], dtype=np.float32)[0, :, 0]
    A = np.zeros((CP, 4, CN, CP, 4), dtype=np.float32)
    A[np.arange(CP), :, :, np.arange(CP), :] = W0.transpose(1, 3, 0, 2) / 9.0
    A = A.reshape(128, CN * CP * 4)
    bv_rep = np.broadcast_to(bv.reshape(1, CNM), (128, CNM)).copy()
    ba_rep = np.broadcast_to(ba.reshape(1, CN), (128, CN)).copy()
    ident = np.eye(128, dtype=np.float32)
    poseF = inputPose.reshape(B, H, W, CPM)
    in_maps = [{
        "poseR": poseF[IMGS * d:IMGS * (d + 1)],
        "actR": inputActivation[IMGS * d:IMGS * (d + 1)],
        "Abd": A, "bvD": bv_rep, "baD": ba_rep, "idD": ident,
    } for d in range(NCORES)]
    nc = _build(float(np.asarray(inputs["inverse_temperature"])),
                float(np.asarray(inputs["pi"])))
    times = []
    for i in range(reps + 1):
        t0 = time.perf_counter()
        run_bass_kernel_spmd(nc, in_maps, list(range(NCORES)))
        t1 = time.perf_counter()
        if i > 0:  # skip compile call
            times.append(t1 - t0)
    return times


if __name__ == "__main__":
    rng = np.random.default_rng(0)
    ins = {
        "inputPose": rng.standard_normal((B, H, W, CP, 4, 4), dtype=np.float32),
        "inputActivation": rng.random((B, H, W, CP), dtype=np.float32),
        "weightMatrix": rng.standard_normal((1, CN, CP, 4, 4), dtype=np.float32),
        "Bv": np.full((1, CN, 1, 16), 0.1, np.float32),
        "Ba": np.full((1, CN, 1), 0.1, np.float32),
        "height": H, "width": W, "epsilon": 1e-5,
        "inverse_temperature": 1.0, "pi": 3.141592653589793,
    }
    m, a = kernel(**ins)
    print("mean", m.shape, m.dtype, "act", a.shape, a.dtype)


# revision 11
# speedup vs baseline: 439.5608x; 439.5608x over previous
"""ConvCaps EM-routing kernel for Trainium2 (8 NeuronCores, data-parallel over N).

Strategy:
  - N = B*Ho*Wo = 4096 positions; 512 per core (2 images), 4 chunks of 128
    (n on partitions); free dim holds (c, p, m) = 32*32*16.
  - vote[n, c, p, m] built on TensorE via block-diagonal weights
    (pool scale 1/9 folded in, contraction over (p_in, j) = 128 partitions).
  - 3 EM iterations unrolled, all fp32 (the EM fixed point amplifies 16-bit
    quantization ~100x, so no bf16/fp16 anywhere on the routing path).
    Big tensor work is split into independent c-quarters so VectorE and
    ScalarE passes of different quarters overlap. Transcendentals use only
    the ln/exp ACT table set (sqrt = exp(0.5 ln), rsqrt = exp(-0.5 ln),
    sigmoid = 1/(1+exp(-x))). The last iteration's E-step is dead code and
    skipped; its M-step keeps full precision for the outputs.
"""

import sys
import numpy as np

sys.path.insert(0, "/opt/trn_rl_repo")

import concourse.bass as bass
import concourse.bacc as bacc
import concourse.mybir as mybir
from concourse.tile import TileContext
from concourse.bass_utils import run_bass_kernel_spmd

F32 = mybir.dt.float32
Alu = mybir.AluOpType
Act = mybir.ActivationFunctionType
AxX = mybir.AxisListType.X

B, H, W, CP, CN = 16, 18, 18, 32, 32
Ho, Wo = 16, 16
Ntot = B * Ho * Wo               # 4096
NCORES = 8
NLOC = Ntot // NCORES            # 512
IMGS = B // NCORES               # 2
CHUNKS = NLOC // 128             # 4
M = 16
CPM = CP * M                     # 512
CNM = CN * M                     # 512
VSZ = CN * CP * M                # 16384
NQ = 4                           # c-quarters
CQ = CN // NQ                    # 8 c's per quarter


def _build(inv_temp: float, pi: float, reps: int = 1):
    nc = bacc.Bacc(None, target_bir_lowering=False, debug=False, num_devices=NCORES)

    poseR = nc.dram_tensor("poseR", [IMGS, H, W, CPM], F32, kind="ExternalInput").ap()
    actR = nc.dram_tensor("actR", [IMGS, H, W, CP], F32, kind="ExternalInput").ap()
    Abd = nc.dram_tensor("Abd", [128, CN * CP * 4], F32, kind="ExternalInput").ap()
    bvD = nc.dram_tensor("bvD", [128, CNM], F32, kind="ExternalInput").ap()
    baD = nc.dram_tensor("baD", [128, CN], F32, kind="ExternalInput").ap()
    idD = nc.dram_tensor("idD", [128, 128], F32, kind="ExternalInput").ap()
    meanO = nc.dram_tensor("meanO", [NLOC, CNM], F32, kind="ExternalOutput").ap()
    actO = nc.dram_tensor("actO", [NLOC, CN], F32, kind="ExternalOutput").ap()

    with TileContext(nc) as tc:
        with tc.tile_pool(name="const", bufs=1) as cpool, \
             tc.tile_pool(name="big", bufs=1) as bigp, \
             tc.tile_pool(name="pose", bufs=2) as posep, \
             tc.tile_pool(name="sm", bufs=1) as smp, \
             tc.tile_pool(name="psT", bufs=2, space="PSUM") as psT, \
             tc.tile_pool(name="psMM", bufs=3, space="PSUM") as psMM:

            Abd_t = cpool.tile([128, CN * CP * 4], F32)
            nc.sync.dma_start(Abd_t[:], Abd[:])
            bv_t = cpool.tile([128, CNM], F32)
            nc.sync.dma_start(bv_t[:], bvD[:])
            ba_t = cpool.tile([128, CN], F32)
            nc.sync.dma_start(ba_t[:], baD[:])
            ident = cpool.tile([128, 128], F32)
            nc.sync.dma_start(ident[:], idD[:])
            bias5 = cpool.tile([128, 1], F32)
            nc.vector.memset(bias5[:], 1e-5)
            bias6 = cpool.tile([128, 1], F32)
            nc.vector.memset(bias6[:], 1e-6)

            for chunk in [c for _ in range(reps) for c in range(CHUNKS)]:
                ib, ih = chunk // 2, chunk % 2
                h0 = 8 * ih

                # ---- pooling: 9 shifted loads + adds
                acc = posep.tile([128, CPM], F32, tag="acc", bufs=1)
                acca = posep.tile([128, CP], F32, tag="acca", bufs=1)
                for t, (dh, dw) in enumerate((d, w) for d in range(3) for w in range(3)):
                    ps_src = poseR[ib, h0 + dh:h0 + dh + 8, dw:dw + 16, :]
                    ac_src = actR[ib, h0 + dh:h0 + dh + 8, dw:dw + 16, :]
                    if t == 0:
                        nc.sync.dma_start(acc[:], ps_src)
                        nc.sync.dma_start(acca[:], ac_src)
                    else:
                        pl = posep.tile([128, CPM], F32, tag="pl")
                        nc.sync.dma_start(pl[:], ps_src)
                        nc.vector.tensor_tensor(acc[:], acc[:], pl[:], Alu.add)
                        al = posep.tile([128, CP], F32, tag="al")
                        nc.sync.dma_start(al[:], ac_src)
                        nc.vector.tensor_tensor(acca[:], acca[:], al[:], Alu.add)
                a_in = smp.tile([128, CP], F32, tag="a_in")
                nc.vector.tensor_scalar_mul(a_in[:], acca[:], 1.0 / 9.0)
                R0 = smp.tile([128, CP], F32, tag="R0")
                nc.vector.tensor_scalar_mul(R0[:], acca[:], 1.0 / (9.0 * CN))

                # ---- poseT[(p,j), n] per k
                poseT = posep.tile([128, 512], F32, tag="poseT", bufs=1)
                acc_v = acc[:].rearrange("n (pj k) -> n k pj", k=4)
                for k in range(4):
                    ks = posep.tile([128, 128], F32, tag="ks")
                    nc.scalar.copy(ks[:], acc_v[:, k, :])
                    pst = psT.tile([128, 128], F32)
                    nc.tensor.transpose(pst[:], ks[:], ident[:])
                    nc.scalar.copy(poseT[:, 128 * k:128 * (k + 1)], pst[:])

                # ---- vote via block-diagonal matmuls (evict on ACT + DVE)
                vote = bigp.tile([128, VSZ], F32, tag="vote")
                vote_r = vote[:].rearrange("n (c p i k) -> n c p i k", c=CN, p=CP, i=4)
                for k in range(4):
                    for half in range(4):
                        pm = psMM.tile([128, 1024], F32)
                        for q in range(2):
                            cg = half * 2 + q
                            nc.tensor.matmul(
                                pm[:, 512 * q:512 * (q + 1)],
                                poseT[:, 128 * k:128 * (k + 1)],
                                Abd_t[:, 512 * cg:512 * (cg + 1)],
                                start=True, stop=True)
                        c0 = half * 8
                        dst = vote_r[:, c0:c0 + 8, :, :, k]
                        src = pm[:].rearrange("n (c p i) -> n c p i", c=8, p=CP)
                        if half % 2 == 0:
                            nc.scalar.copy(dst, src)
                        else:
                            nc.vector.tensor_copy(dst, src)

                vote_v = vote[:].rearrange("n (c p m) -> n c p m", c=CN, p=CP)

                scr = bigp.tile([128, VSZ // NQ], F32, tag="scr")
                scr2 = bigp.tile([128, VSZ // NQ], F32, tag="scr2")
                scrE = bigp.tile([128, VSZ // NQ], F32, tag="scrE")

                R = smp.tile([128, CN * CP], F32, tag="R")
                S1 = smp.tile([128, CNM], F32, tag="S1")
                S2 = smp.tile([128, CNM], F32, tag="S2")
                mean = smp.tile([128, CNM], F32, tag="mean")
                var = smp.tile([128, CNM], F32, tag="var")
                t1 = smp.tile([128, CNM], F32, tag="t1")
                t2 = smp.tile([128, CNM], F32, tag="t2")
                b1 = smp.tile([128, CNM], F32, tag="b1")
                sE = smp.tile([128, CN * CP], F32, tag="sE")
                pu = smp.tile([128, CN * CP], F32, tag="pu")
                sum_cost = smp.tile([128, CN], F32, tag="sum_cost")
                sd = smp.tile([128, CN], F32, tag="sd")
                sq = smp.tile([128, CN], F32, tag="sq")
                u1 = smp.tile([128, CN], F32, tag="u1")
                act = smp.tile([128, CN], F32, tag="act")
                rsum = smp.tile([128, CN], F32, tag="rsum")
                i5 = smp.tile([128, CN], F32, tag="i5")
                i3 = smp.tile([128, CN], F32, tag="i3")
                varsum = smp.tile([128, CN], F32, tag="varsum")
                denom = smp.tile([128, CN], F32, tag="denom")
                adr = smp.tile([128, CN], F32, tag="adr")
                apsum = smp.tile([128, CP], F32, tag="apsum")
                rra = smp.tile([128, CP], F32, tag="rra")
                sc1 = smp.tile([128, 1], F32, tag="sc1")
                sc2 = smp.tile([128, 1], F32, tag="sc2")
                sc3 = smp.tile([128, 1], F32, tag="sc3")

                cm = lambda ap: ap.rearrange("n (c m) -> n c m", c=CN)
                cp_ = lambda ap: ap.rearrange("n (c p) -> n c p", c=CN)

                for t in range(3):
                    it_val = inv_temp + t

                    # ============== M-step: S1, S2 per c-quarter ==============
                    for h in range(NQ):
                        cs = slice(h * CQ, (h + 1) * CQ)
                        vf = vote_v[:, cs, :, :]
                        sA = scr[:, 0:CQ * CP * M].rearrange(
                            "n (c p m) -> n c p m", c=CQ, p=CP)
                        sB = scr2[:, 0:CQ * CP * M].rearrange(
                            "n (c p m) -> n c p m", c=CQ, p=CP)
                        if t == 0:
                            wv = R0[:].unsqueeze(1).unsqueeze(3) \
                                .broadcast_to([128, CQ, CP, M])
                        else:
                            wv = cp_(R[:])[:, cs, :].unsqueeze(3) \
                                .broadcast_to([128, CQ, CP, M])
                        nc.vector.tensor_tensor(sA, vf, wv, Alu.mult)
                        nc.vector.tensor_tensor(sB, sA, vf, Alu.mult)
                        nc.vector.tensor_reduce(
                            S1[:, h * CQ * M:(h + 1) * CQ * M]
                            .rearrange("n (c m) -> n c m", c=CQ),
                            sA.transpose([0, 1, 3, 2]), axis=AxX, op=Alu.add)
                        nc.vector.tensor_reduce(
                            S2[:, h * CQ * M:(h + 1) * CQ * M]
                            .rearrange("n (c m) -> n c m", c=CQ),
                            sB.transpose([0, 1, 3, 2]), axis=AxX, op=Alu.add)

                    # ============== stats: mean, var ==============
                    if t == 0:
                        nc.vector.tensor_reduce(sc1[:], R0[:], axis=AxX, op=Alu.add)
                        nc.vector.tensor_scalar_add(sc2[:], sc1[:], 1e-5)
                        nc.vector.reciprocal(sc2[:], sc2[:])
                        nc.vector.tensor_scalar_mul(mean[:], S1[:], sc2[:, 0:1])
                        nc.vector.tensor_scalar_add(sc3[:], sc1[:], 1e-3)
                        nc.vector.reciprocal(sc3[:], sc3[:])
                        nc.vector.tensor_tensor(t1[:], mean[:], S1[:], Alu.mult)
                        nc.vector.scalar_tensor_tensor(
                            t2[:], mean[:], sc1[:, 0:1], mean[:], Alu.mult, Alu.mult)
                        nc.vector.scalar_tensor_tensor(
                            t1[:], t1[:], -2.0, S2[:], Alu.mult, Alu.add)
                        nc.vector.tensor_tensor(t1[:], t1[:], t2[:], Alu.add)
                        nc.vector.tensor_scalar_mul(var[:], t1[:], sc3[:, 0:1])
                    else:
                        nc.vector.tensor_reduce(rsum[:], cp_(R[:]), axis=AxX, op=Alu.add)
                        nc.vector.tensor_scalar_add(i5[:], rsum[:], 1e-5)
                        nc.vector.reciprocal(i5[:], i5[:])
                        nc.vector.tensor_scalar_add(i3[:], rsum[:], 1e-3)
                        nc.vector.reciprocal(i3[:], i3[:])
                        i5_b = i5[:].unsqueeze(2).broadcast_to([128, CN, M])
                        i3_b = i3[:].unsqueeze(2).broadcast_to([128, CN, M])
                        rsum_b = rsum[:].unsqueeze(2).broadcast_to([128, CN, M])
                        nc.vector.tensor_tensor(cm(mean[:]), cm(S1[:]), i5_b, Alu.mult)
                        # var = (S2 - mean*(2*S1 - mean*rsum)) / (rsum+1e-3)
                        nc.vector.tensor_tensor(cm(t2[:]), cm(mean[:]), rsum_b, Alu.mult)
                        nc.vector.scalar_tensor_tensor(
                            t1[:], S1[:], 2.0, t2[:], Alu.mult, Alu.subtract)
                        nc.vector.tensor_tensor(t1[:], mean[:], t1[:], Alu.mult)
                        nc.vector.scalar_tensor_tensor(
                            t1[:], t1[:], -1.0, S2[:], Alu.mult, Alu.add)
                        nc.vector.tensor_tensor(cm(var[:]), cm(t1[:]), i3_b, Alu.mult)

                    # ============== cost -> act_out ==============
                    nc.scalar.activation(t1[:], var[:], Act.Ln, bias=bias6[:, 0:1])
                    nc.scalar.activation(t2[:], t1[:], Act.Exp, scale=0.5)
                    nc.scalar.activation(t1[:], t2[:], Act.Ln, bias=bias5[:, 0:1])
                    nc.vector.tensor_tensor(t1[:], t1[:], bv_t[:], Alu.add)
                    nc.vector.tensor_reduce(sum_cost[:], cm(t1[:]), axis=AxX, op=Alu.add)
                    if t == 0:
                        nc.vector.tensor_scalar_mul(sum_cost[:], sum_cost[:], sc1[:, 0:1])
                    else:
                        nc.vector.tensor_tensor(sum_cost[:], sum_cost[:], rsum[:], Alu.mult)
                    nc.vector.tensor_reduce(sc1[:], sum_cost[:], axis=AxX, op=Alu.add)
                    nc.vector.tensor_scalar_mul(sc1[:], sc1[:], 1.0 / CN)   # scm
                    nc.vector.tensor_scalar_sub(sd[:], sum_cost[:], sc1[:, 0:1])
                    nc.vector.tensor_tensor(sq[:], sd[:], sd[:], Alu.mult)
                    nc.vector.tensor_reduce(sc2[:], sq[:], axis=AxX, op=Alu.add)
                    nc.scalar.activation(sc2[:], sc2[:], Act.Ln, scale=1.0 / CN,
                                         bias=bias5[:, 0:1])
                    nc.scalar.activation(sc2[:], sc2[:], Act.Exp, scale=0.5)  # std
                    nc.vector.tensor_scalar_add(sc2[:], sc2[:], 1e-5)
                    nc.vector.reciprocal(sc2[:], sc2[:])
                    nc.vector.tensor_scalar_mul(sc2[:], sc2[:], -1.0)
                    nc.vector.scalar_tensor_tensor(
                        u1[:], sd[:], sc2[:, 0:1], ba_t[:], Alu.mult, Alu.add)
                    nc.scalar.activation(act[:], u1[:], Act.Exp, scale=-it_val)
                    nc.vector.tensor_scalar_add(act[:], act[:], 1.0)
                    nc.vector.reciprocal(act[:], act[:])

                    if t == 2:
                        nc.sync.dma_start(meanO[128 * chunk:128 * (chunk + 1), :], mean[:])
                        nc.sync.dma_start(actO[128 * chunk:128 * (chunk + 1), :], act[:])
                        continue

                    # ============== E-step ==============
                    nc.scalar.activation(t1[:], var[:], Act.Ln, scale=2.0,
                                         bias=bias5[:, 0:1])
                    nc.scalar.activation(b1[:], t1[:], Act.Exp, scale=-0.5)
                    for h in range(NQ):
                        cs = slice(h * CQ, (h + 1) * CQ)
                        vf = vote_v[:, cs, :, :]
                        sA = scrE[:, 0:CQ * CP * M].rearrange(
                            "n (c p m) -> n c p m", c=CQ, p=CP)
                        mb = cm(mean[:])[:, cs, :].unsqueeze(2) \
                            .broadcast_to([128, CQ, CP, M])
                        bb = cm(b1[:])[:, cs, :].unsqueeze(2) \
                            .broadcast_to([128, CQ, CP, M])
                        nc.vector.tensor_tensor(sA, vf, mb, Alu.subtract)
                        nc.vector.tensor_tensor(sA, sA, bb, Alu.mult)
                        nc.scalar.activation(sA, sA, Act.Square)
                        nc.vector.tensor_reduce(
                            sE[:, h * CQ * CP:(h + 1) * CQ * CP]
                            .rearrange("n (c p) -> n c p", c=CQ),
                            sA, axis=AxX, op=Alu.add)

                    nc.vector.tensor_reduce(varsum[:], cm(var[:]), axis=AxX, op=Alu.add)
                    nc.scalar.activation(denom[:], varsum[:], Act.Ln, scale=2.0 * pi)
                    nc.scalar.activation(denom[:], denom[:], Act.Exp, scale=0.5)
                    nc.vector.tensor_scalar_add(denom[:], denom[:], 1e-4)
                    nc.vector.reciprocal(denom[:], denom[:])
                    nc.vector.tensor_tensor(adr[:], act[:], denom[:], Alu.mult)

                    nc.scalar.activation(pu[:], sE[:], Act.Exp, scale=-1.0)
                    adr_b = adr[:].unsqueeze(2).broadcast_to([128, CN, CP])
                    nc.vector.tensor_tensor(cp_(pu[:]), cp_(pu[:]), adr_b, Alu.mult)
                    nc.vector.tensor_reduce(
                        apsum[:], cp_(pu[:]).transpose([0, 2, 1]), axis=AxX, op=Alu.add)
                    nc.vector.tensor_scalar_add(apsum[:], apsum[:], 1e-5)
                    nc.vector.reciprocal(apsum[:], apsum[:])
                    nc.vector.tensor_tensor(rra[:], apsum[:], a_in[:], Alu.mult)
                    rra_b = rra[:].unsqueeze(1).broadcast_to([128, CN, CP])
                    nc.vector.tensor_tensor(cp_(R[:]), cp_(pu[:]), rra_b, Alu.mult)

    nc.compile()
    return nc


def _prep(inputs):
    inputPose = np.asarray(inputs["inputPose"], dtype=np.float32)
    inputActivation = np.asarray(inputs["inputActivation"], dtype=np.float32)
    W0 = np.asarray(inputs["weightMatrix"], dtype=np.float32)[0]
    bv = np.asarray(inputs["Bv"], dtype=np.float32)[0, :, 0, :]
    ba = np.asarray(inputs["Ba"], dtype=np.float32)[0, :, 0]
    A = np.zeros((CP, 4, CN, CP, 4), dtype=np.float32)
    A[np.arange(CP), :, :, np.arange(CP), :] = W0.transpose(1, 3, 0, 2) / 9.0
    A = A.reshape(128, CN * CP * 4)
    bv_rep = np.broadcast_to(bv.reshape(1, CNM), (128, CNM)).copy()
    ba_rep = np.broadcast_to(ba.reshape(1, CN), (128, CN)).copy()
    ident = np.eye(128, dtype=np.float32)
    poseF = inputPose.reshape(B, H, W, CPM)
    return [{
        "poseR": poseF[IMGS * d:IMGS * (d + 1)],
        "actR": inputActivation[IMGS * d:IMGS * (d + 1)],
        "Abd": A, "bvD": bv_rep, "baD": ba_rep, "idD": ident,
    } for d in range(NCORES)]


def kernel(inputPose, inputActivation, weightMatrix, Bv, Ba,
           height, width, epsilon, inverse_temperature, pi):
    inputs = dict(inputPose=inputPose, inputActivation=inputActivation,
                  weightMatrix=weightMatrix, Bv=Bv, Ba=Ba)
    in_maps = _prep(inputs)
    nc = _build(float(np.asarray(inverse_temperature)), float(np.asarray(pi)))
    res = run_bass_kernel_spmd(nc, in_maps, list(range(NCORES)))
    mean = np.concatenate([r["meanO"] for r in res.results], axis=0).reshape(Ntot, CN, M)
    actv = np.concatenate([r["actO"] for r in res.results], axis=0)
    return mean, actv


def bench(inputs, reps=10):
    import time
    in_maps = _prep(inputs)
    nc = _build(float(np.asarray(inputs["inverse_temperature"])),
                float(np.asarray(inputs["pi"])))
    times = []
    for i in range(reps + 1):
        t0 = time.perf_counter()
        run_bass_kernel_spmd(nc, in_maps, list(range(NCORES)))
        t1 = time.perf_counter()
        if i > 0:
            times.append(t1 - t0)
    return times


# revision 19
# speedup vs baseline: 462.0246x; 1.0511x over previous
"""ConvCaps EM-routing kernel for Trainium2 (8 NeuronCores, data-parallel over N).

Strategy:
  - N = B*Ho*Wo = 4096 positions; 512 per core (2 images), 4 chunks of 128
    (n on partitions); free dim holds (c, p, m) = 32*32*16.
  - vote[n, c, p, m] built on TensorE via block-diagonal weights
    (pool scale 1/9 folded in, contraction over (p_in, j) = 128 partitions).
  - 3 EM iterations unrolled, all fp32 (the EM fixed point amplifies 16-bit
    quantization ~100x, so no bf16/fp16 anywhere on the routing path).
    Big tensor work is split into independent c-quarters so VectorE and
    ScalarE passes of different quarters overlap. Transcendentals use only
    the ln/exp ACT table set (sqrt = exp(0.5 ln), rsqrt = exp(-0.5 ln),
    sigmoid = 1/(1+exp(-x))). The last iteration's E-step is dead code and
    skipped; its M-step keeps full precision for the outputs.
"""

import sys
import numpy as np

sys.path.insert(0, "/opt/trn_rl_repo")

import concourse.bass as bass
import concourse.bacc as bacc
import concourse.mybir as mybir
from concourse.tile import TileContext
from concourse.bass_utils import run_bass_kernel_spmd

# The act-table-load pass greedily picks the first set containing each
# activation function, alternating exp_and_others <-> natural_log (one
# ~2.7us table load per transcendental group). Hide Exp/Ln from every set
# except the combined natural_log_exp_and_others so a single table serves
# the whole kernel. Indices (act_func_set_id) are preserved.
if getattr(bacc.get_activation_tables, "_convcaps_patched", False):
    _orig_get_tables = bacc.get_activation_tables._convcaps_orig
else:
    _orig_get_tables = bacc.get_activation_tables

def _patched_get_tables(arch):
    tabs = {k: set(v) for k, v in _orig_get_tables(arch).items()}
    comb = "natural_log_exp_and_others"
    if comb in tabs:
        exp_ln = {f for f in tabs[comb]
                  if getattr(f, "name", str(f)).lower() in ("exp", "ln")}
        for name, funcs in tabs.items():
            if name != comb:
                funcs -= exp_ln
    return tabs

_patched_get_tables._convcaps_patched = True
_patched_get_tables._convcaps_orig = _orig_get_tables
bacc.get_activation_tables = _patched_get_tables

F32 = mybir.dt.float32
Alu = mybir.AluOpType
Act = mybir.ActivationFunctionType
AxX = mybir.AxisListType.X

B, H, W, CP, CN = 16, 18, 18, 32, 32
Ho, Wo = 16, 16
Ntot = B * Ho * Wo               # 4096
NCORES = 8
NLOC = Ntot // NCORES            # 512
IMGS = B // NCORES               # 2
CHUNKS = NLOC // 128             # 4
M = 16
CPM = CP * M                     # 512
CNM = CN * M                     # 512
VSZ = CN * CP * M                # 16384
import os
NQ = int(os.environ.get("K_NQ", "4"))            # c-pieces for big passes
CQ = CN // NQ
EVICT_SPLIT = os.environ.get("K_EVSPLIT", "0") == "1"
POOL_ON_ACT = os.environ.get("K_POOLACT", "0") == "1" 


def _build(inv_temp: float, pi: float, reps: int = 1):
    nc = bacc.Bacc(None, target_bir_lowering=False, debug=False, num_devices=NCORES)

    poseR = nc.dram_tensor("poseR", [IMGS, H, W, CPM], F32, kind="ExternalInput").ap()
    actR = nc.dram_tensor("actR", [IMGS, H, W, CP], F32, kind="ExternalInput").ap()
    Abd = nc.dram_tensor("Abd", [128, CN * CP * 4], F32, kind="ExternalInput").ap()
    bvD = nc.dram_tensor("bvD", [128, CNM], F32, kind="ExternalInput").ap()
    baD = nc.dram_tensor("baD", [128, CN], F32, kind="ExternalInput").ap()
    idD = nc.dram_tensor("idD", [128, 128], F32, kind="ExternalInput").ap()
    meanO = nc.dram_tensor("meanO", [NLOC, CNM], F32, kind="ExternalOutput").ap()
    actO = nc.dram_tensor("actO", [NLOC, CN], F32, kind="ExternalOutput").ap()

    with TileContext(nc) as tc:
        with tc.tile_pool(name="const", bufs=1) as cpool, \
             tc.tile_pool(name="big", bufs=1) as bigp, \
             tc.tile_pool(name="pose", bufs=2) as posep, \
             tc.tile_pool(name="sm", bufs=1) as smp, \
             tc.tile_pool(name="psT", bufs=2, space="PSUM") as psT, \
             tc.tile_pool(name="psMM", bufs=3, space="PSUM") as psMM:

            Abd_t = cpool.tile([128, CN * CP * 4], F32)
            nc.sync.dma_start(Abd_t[:], Abd[:])
            bv_t = cpool.tile([128, CNM], F32)
            nc.sync.dma_start(bv_t[:], bvD[:])
            ba_t = cpool.tile([128, CN], F32)
            nc.sync.dma_start(ba_t[:], baD[:])
            ident = cpool.tile([128, 128], F32)
            nc.sync.dma_start(ident[:], idD[:])
            bias5 = cpool.tile([128, 1], F32)
            nc.vector.memset(bias5[:], 1e-5)
            bias6 = cpool.tile([128, 1], F32)
            nc.vector.memset(bias6[:], 1e-6)

            for chunk in [c for _ in range(reps) for c in range(CHUNKS)]:
                ib, ih = chunk // 2, chunk % 2
                h0 = 8 * ih

                # ---- pooling: 9 shifted loads + adds
                acc = posep.tile([128, CPM], F32, tag="acc", bufs=1)
                acca = posep.tile([128, CP], F32, tag="acca", bufs=1)
                for t, (dh, dw) in enumerate((d, w) for d in range(3) for w in range(3)):
                    ps_src = poseR[ib, h0 + dh:h0 + dh + 8, dw:dw + 16, :]
                    ac_src = actR[ib, h0 + dh:h0 + dh + 8, dw:dw + 16, :]
                    if t == 0:
                        nc.sync.dma_start(acc[:], ps_src)
                        nc.sync.dma_start(acca[:], ac_src)
                    else:
                        pl = posep.tile([128, CPM], F32, tag="pl")
                        nc.sync.dma_start(pl[:], ps_src)
                        nc.gpsimd.tensor_tensor(acc[:], acc[:], pl[:], Alu.add)
                        al = posep.tile([128, CP], F32, tag="al")
                        nc.sync.dma_start(al[:], ac_src)
                        nc.gpsimd.tensor_tensor(acca[:], acca[:], al[:], Alu.add)
                a_in = smp.tile([128, CP], F32, tag="a_in")
                nc.vector.tensor_scalar_mul(a_in[:], acca[:], 1.0 / 9.0)
                R0 = smp.tile([128, CP], F32, tag="R0")
                nc.vector.tensor_scalar_mul(R0[:], acca[:], 1.0 / (9.0 * CN))

                # ---- poseT[(p,j), n] per k
                poseT = posep.tile([128, 512], F32, tag="poseT", bufs=1)
                acc_v = acc[:].rearrange("n (pj k) -> n k pj", k=4)
                for k in range(4):
                    ks = posep.tile([128, 128], F32, tag="ks")
                    nc.scalar.copy(ks[:], acc_v[:, k, :])
                    pst = psT.tile([128, 128], F32)
                    nc.tensor.transpose(pst[:], ks[:], ident[:])
                    nc.scalar.copy(poseT[:, 128 * k:128 * (k + 1)], pst[:])

                # ---- vote via block-diagonal matmuls (evict on ACT + DVE)
                vote = bigp.tile([128, VSZ], F32, tag="vote")
                vote_r = vote[:].rearrange("n (c p i k) -> n c p i k", c=CN, p=CP, i=4)
                for k in range(4):
                    for half in range(4):
                        pm = psMM.tile([128, 1024], F32)
                        for q in range(2):
                            cg = half * 2 + q
                            nc.tensor.matmul(
                                pm[:, 512 * q:512 * (q + 1)],
                                poseT[:, 128 * k:128 * (k + 1)],
                                Abd_t[:, 512 * cg:512 * (cg + 1)],
                                start=True, stop=True)
                        c0 = half * 8
                        dst = vote_r[:, c0:c0 + 8, :, :, k]
                        src = pm[:].rearrange("n (c p i) -> n c p i", c=8, p=CP)
                        if EVICT_SPLIT and half % 2 == 1:
                            nc.vector.tensor_copy(dst, src)
                        else:
                            nc.scalar.copy(dst, src)

                vote_v = vote[:].rearrange("n (c p m) -> n c p m", c=CN, p=CP)

                scr = bigp.tile([128, VSZ // NQ], F32, tag="scr")
                scr2 = bigp.tile([128, VSZ // NQ], F32, tag="scr2")
                scrE = bigp.tile([128, VSZ // NQ], F32, tag="scrE")

                R = smp.tile([128, CN * CP], F32, tag="R")
                S1 = smp.tile([128, CNM], F32, tag="S1")
                S2 = smp.tile([128, CNM], F32, tag="S2")
                mean = smp.tile([128, CNM], F32, tag="mean")
                var = smp.tile([128, CNM], F32, tag="var")
                t1 = smp.tile([128, CNM], F32, tag="t1")
                t2 = smp.tile([128, CNM], F32, tag="t2")
                b1 = smp.tile([128, CNM], F32, tag="b1")
                sE = smp.tile([128, CN * CP], F32, tag="sE")
                pu = smp.tile([128, CN * CP], F32, tag="pu")
                sum_cost = smp.tile([128, CN], F32, tag="sum_cost")
                sd = smp.tile([128, CN], F32, tag="sd")
                sq = smp.tile([128, CN], F32, tag="sq")
                u1 = smp.tile([128, CN], F32, tag="u1")
                act = smp.tile([128, CN], F32, tag="act")
                rsum = smp.tile([128, CN], F32, tag="rsum")
                i5 = smp.tile([128, CN], F32, tag="i5")
                i3 = smp.tile([128, CN], F32, tag="i3")
                varsum = smp.tile([128, CN], F32, tag="varsum")
                denom = smp.tile([128, CN], F32, tag="denom")
                adr = smp.tile([128, CN], F32, tag="adr")
                apsum = smp.tile([128, CP], F32, tag="apsum")
                rra = smp.tile([128, CP], F32, tag="rra")
                sc1 = smp.tile([128, 1], F32, tag="sc1")
                sc2 = smp.tile([128, 1], F32, tag="sc2")
                sc3 = smp.tile([128, 1], F32, tag="sc3")

                cm = lambda ap: ap.rearrange("n (c m) -> n c m", c=CN)
                cp_ = lambda ap: ap.rearrange("n (c p) -> n c p", c=CN)

                for t in range(3):
                    it_val = inv_temp + t

                    # ============== M-step: S1, S2 per c-quarter ==============
                    for h in range(NQ):
                        cs = slice(h * CQ, (h + 1) * CQ)
                        vf = vote_v[:, cs, :, :]
                        sA = scr[:, 0:CQ * CP * M].rearrange(
                            "n (c p m) -> n c p m", c=CQ, p=CP)
                        sB = scr2[:, 0:CQ * CP * M].rearrange(
                            "n (c p m) -> n c p m", c=CQ, p=CP)
                        if t == 0:
                            wv = R0[:].unsqueeze(1).unsqueeze(3) \
                                .broadcast_to([128, CQ, CP, M])
                        else:
                            wv = cp_(R[:])[:, cs, :].unsqueeze(3) \
                                .broadcast_to([128, CQ, CP, M])
                        nc.vector.tensor_tensor(sA, vf, wv, Alu.mult)
                        nc.vector.tensor_tensor(sB, sA, vf, Alu.mult)
                        nc.vector.tensor_reduce(
                            S1[:, h * CQ * M:(h + 1) * CQ * M]
                            .rearrange("n (c m) -> n c m", c=CQ),
                            sA.transpose([0, 1, 3, 2]), axis=AxX, op=Alu.add)
                        nc.vector.tensor_reduce(
                            S2[:, h * CQ * M:(h + 1) * CQ * M]
                            .rearrange("n (c m) -> n c m", c=CQ),
                            sB.transpose([0, 1, 3, 2]), axis=AxX, op=Alu.add)

                    # ============== stats: mean, var ==============
                    if t == 0:
                        nc.vector.tensor_reduce(sc1[:], R0[:], axis=AxX, op=Alu.add)
                        nc.vector.tensor_scalar_add(sc2[:], sc1[:], 1e-5)
                        nc.vector.reciprocal(sc2[:], sc2[:])
                        nc.scalar.mul(mean[:], S1[:], sc2[:, 0:1])
                        nc.vector.tensor_scalar_add(sc3[:], sc1[:], 1e-3)
                        nc.vector.reciprocal(sc3[:], sc3[:])
                        nc.vector.tensor_tensor(t1[:], mean[:], S1[:], Alu.mult)
                        nc.vector.scalar_tensor_tensor(
                            t2[:], mean[:], sc1[:, 0:1], mean[:], Alu.mult, Alu.mult)
                        nc.vector.scalar_tensor_tensor(
                            t1[:], t1[:], -2.0, S2[:], Alu.mult, Alu.add)
                        nc.vector.tensor_tensor(t1[:], t1[:], t2[:], Alu.add)
                        nc.scalar.mul(var[:], t1[:], sc3[:, 0:1])
                    else:
                        nc.vector.tensor_reduce(rsum[:], cp_(R[:]), axis=AxX, op=Alu.add)
                        nc.vector.tensor_scalar_add(i5[:], rsum[:], 1e-5)
                        nc.vector.reciprocal(i5[:], i5[:])
                        nc.vector.tensor_scalar_add(i3[:], rsum[:], 1e-3)
                        nc.vector.reciprocal(i3[:], i3[:])
                        i5_b = i5[:].unsqueeze(2).broadcast_to([128, CN, M])
                        i3_b = i3[:].unsqueeze(2).broadcast_to([128, CN, M])
                        rsum_b = rsum[:].unsqueeze(2).broadcast_to([128, CN, M])
                        nc.vector.tensor_tensor(cm(mean[:]), cm(S1[:]), i5_b, Alu.mult)
                        # var = (S2 - mean*(2*S1 - mean*rsum)) / (rsum+1e-3)
                        nc.vector.tensor_tensor(cm(t2[:]), cm(mean[:]), rsum_b, Alu.mult)
                        nc.vector.scalar_tensor_tensor(
                            t1[:], S1[:], 2.0, t2[:], Alu.mult, Alu.subtract)
                        nc.vector.tensor_tensor(t1[:], mean[:], t1[:], Alu.mult)
                        nc.vector.scalar_tensor_tensor(
                            t1[:], t1[:], -1.0, S2[:], Alu.mult, Alu.add)
                        nc.vector.tensor_tensor(cm(var[:]), cm(t1[:]), i3_b, Alu.mult)

                    # ============== cost -> act_out ==============
                    nc.scalar.activation(t1[:], var[:], Act.Ln, bias=bias6[:, 0:1])
                    nc.scalar.activation(t2[:], t1[:], Act.Exp, scale=0.5)
                    nc.scalar.activation(t1[:], t2[:], Act.Ln, bias=bias5[:, 0:1])
                    nc.vector.tensor_tensor(t1[:], t1[:], bv_t[:], Alu.add)
                    nc.vector.tensor_reduce(sum_cost[:], cm(t1[:]), axis=AxX, op=Alu.add)
                    if t == 0:
                        nc.vector.tensor_scalar_mul(sum_cost[:], sum_cost[:], sc1[:, 0:1])
                    else:
                        nc.vector.tensor_tensor(sum_cost[:], sum_cost[:], rsum[:], Alu.mult)
                    nc.vector.tensor_reduce(sc1[:], sum_cost[:], axis=AxX, op=Alu.add)
                    nc.vector.tensor_scalar_mul(sc1[:], sc1[:], 1.0 / CN)   # scm
                    nc.vector.tensor_scalar_sub(sd[:], sum_cost[:], sc1[:, 0:1])
                    nc.vector.tensor_tensor(sq[:], sd[:], sd[:], Alu.mult)
                    nc.vector.tensor_reduce(sc2[:], sq[:], axis=AxX, op=Alu.add)
                    nc.scalar.activation(sc2[:], sc2[:], Act.Ln, scale=1.0 / CN,
                                         bias=bias5[:, 0:1])
                    nc.scalar.activation(sc2[:], sc2[:], Act.Exp, scale=0.5)  # std
                    nc.vector.tensor_scalar_add(sc2[:], sc2[:], 1e-5)
                    nc.vector.reciprocal(sc2[:], sc2[:])
                    nc.vector.tensor_scalar_mul(sc2[:], sc2[:], -1.0)
                    nc.vector.scalar_tensor_tensor(
                        u1[:], sd[:], sc2[:, 0:1], ba_t[:], Alu.mult, Alu.add)
                    nc.scalar.activation(act[:], u1[:], Act.Exp, scale=-it_val)
                    nc.vector.tensor_scalar_add(act[:], act[:], 1.0)
                    nc.vector.reciprocal(act[:], act[:])

                    if t == 2:
                        nc.sync.dma_start(meanO[128 * chunk:128 * (chunk + 1), :], mean[:])
                        nc.sync.dma_start(actO[128 * chunk:128 * (chunk + 1), :], act[:])
                        continue

                    # ============== E-step ==============
                    nc.scalar.activation(t1[:], var[:], Act.Ln, scale=2.0,
                                         bias=bias5[:, 0:1])
                    nc.scalar.activation(b1[:], t1[:], Act.Exp, scale=-0.5)
                    for h in range(NQ):
                        cs = slice(h * CQ, (h + 1) * CQ)
                        vf = vote_v[:, cs, :, :]
                        sA = scrE[:, 0:CQ * CP * M].rearrange(
                            "n (c p m) -> n c p m", c=CQ, p=CP)
                        mb = cm(mean[:])[:, cs, :].unsqueeze(2) \
                            .broadcast_to([128, CQ, CP, M])
                        bb = cm(b1[:])[:, cs, :].unsqueeze(2) \
                            .broadcast_to([128, CQ, CP, M])
                        GPS_E = int(os.environ.get("K_GPS_E", "0"))
                        eng0 = nc.gpsimd if GPS_E >= 1 else nc.vector
                        eng1 = nc.gpsimd if GPS_E >= 2 else nc.vector
                        eng0.tensor_tensor(sA, vf, mb, Alu.subtract)
                        eng1.tensor_tensor(sA, sA, bb, Alu.mult)
                        nc.scalar.activation(sA, sA, Act.Square)
                        nc.vector.tensor_reduce(
                            sE[:, h * CQ * CP:(h + 1) * CQ * CP]
                            .rearrange("n (c p) -> n c p", c=CQ),
                            sA, axis=AxX, op=Alu.add)

                    nc.vector.tensor_reduce(varsum[:], cm(var[:]), axis=AxX, op=Alu.add)
                    nc.scalar.activation(denom[:], varsum[:], Act.Ln, scale=2.0 * pi)
                    nc.scalar.activation(denom[:], denom[:], Act.Exp, scale=0.5)
                    nc.vector.tensor_scalar_add(denom[:], denom[:], 1e-4)
                    nc.vector.reciprocal(denom[:], denom[:])
                    nc.vector.tensor_tensor(adr[:], act[:], denom[:], Alu.mult)

                    nc.scalar.activation(pu[:], sE[:], Act.Exp, scale=-1.0)
                    adr_b = adr[:].unsqueeze(2).broadcast_to([128, CN, CP])
                    nc.vector.tensor_tensor(cp_(pu[:]), cp_(pu[:]), adr_b, Alu.mult)
                    nc.vector.tensor_reduce(
                        apsum[:], cp_(pu[:]).transpose([0, 2, 1]), axis=AxX, op=Alu.add)
                    nc.vector.tensor_scalar_add(apsum[:], apsum[:], 1e-5)
                    nc.vector.reciprocal(apsum[:], apsum[:])
                    nc.vector.tensor_tensor(rra[:], apsum[:], a_in[:], Alu.mult)
                    rra_b = rra[:].unsqueeze(1).broadcast_to([128, CN, CP])
                    nc.vector.tensor_tensor(cp_(R[:]), cp_(pu[:]), rra_b, Alu.mult)

    nc.compile()
    return nc


def _prep(inputs):
    inputPose = np.asarray(inputs["inputPose"], dtype=np.float32)
    inputActivation = np.asarray(inputs["inputActivation"], dtype=np.float32)
    W0 = np.asarray(inputs["weightMatrix"], dtype=np.float32)[0]
    bv = np.asarray(inputs["Bv"], dtype=np.float32)[0, :, 0, :]
    ba = np.asarray(inputs["Ba"], dtype=np.float32)[0, :, 0]
    A = np.zeros((CP, 4, CN, CP, 4), dtype=np.float32)
    A[np.arange(CP), :, :, np.arange(CP), :] = W0.transpose(1, 3, 0, 2) / 9.0
    A = A.reshape(128, CN * CP * 4)
    bv_rep = np.broadcast_to(bv.reshape(1, CNM), (128, CNM)).copy()
    ba_rep = np.broadcast_to(ba.reshape(1, CN), (128, CN)).copy()
    ident = np.eye(128, dtype=np.float32)
    poseF = inputPose.reshape(B, H, W, CPM)
    return [{
        "poseR": poseF[IMGS * d:IMGS * (d + 1)],
        "actR": inputActivation[IMGS * d:IMGS * (d + 1)],
        "Abd": A, "bvD": bv_rep, "baD": ba_rep, "idD": ident,
    } for d in range(NCORES)]


def kernel(inputPose, inputActivation, weightMatrix, Bv, Ba,
           height, width, epsilon, inverse_temperature, pi):
    inputs = dict(inputPose=inputPose, inputActivation=inputActivation,
                  weightMatrix=weightMatrix, Bv=Bv, Ba=Ba)
    in_maps = _prep(inputs)
    nc = _build(float(np.asarray(inverse_temperature)), float(np.asarray(pi)))
    res = run_bass_kernel_spmd(nc, in_maps, list(range(NCORES)))
    mean = np.concatenate([r["meanO"] for r in res.results], axis=0).reshape(Ntot, CN, M)
    actv = np.concatenate([r["actO"] for r in res.results], axis=0)
    return mean, actv


def bench(inputs, reps=10):
    import time
    in_maps = _prep(inputs)
    nc = _build(float(np.asarray(inputs["inverse_temperature"])),
                float(np.asarray(inputs["pi"])))
    times = []
    for i in range(reps + 1):
        t0 = time.perf_counter()
        run_bass_kernel_spmd(nc, in_maps, list(range(NCORES)))
        t1 = time.perf_counter()
        if i > 0:
            times.append(t1 - t0)
    return times
